# revision 59
# baseline (speedup 1.0000x reference)
"""Trainium2 Bass kernel for nn_AutoregressiveTokenHead (4-layer transformer
decoder with cross-attention + 32k-vocab head), SPMD over 8 NeuronCores.

Sharding: data-parallel over batch (2 batches/core) for the transformer
layers; AllGather of final hidden states; vocab-parallel output head
(4000 vocab columns/core).

Host side: token+position embedding (a gather), LayerNorm gain folding into
the adjacent matmul weights, softmax scale folded into the Q projections,
dtype conversion of weights. All bias inputs are zero for this problem; a
numpy fallback handles the (never exercised) nonzero-bias case.
"""
import sys
sys.path.insert(0, "/opt/trn_rl_repo")

import numpy as np
import ml_dtypes

import concourse.bass as bass
import concourse.bacc as bacc
import concourse.tile as tile
import concourse.mybir as mybir
from concourse.bass_utils import run_bass_kernel_spmd

# ---- problem constants (hardcoded per spec) ----
B, MEM, T = 16, 256, 128
D, H, L, V = 512, 8, 4, 32000
BOS = 32000
HD = D // H            # 64
DF = 4 * D             # 2048
NC = 8                 # cores
BL = B // NC           # 2 batches per core
TOK = BL * T           # 256 tokens per core
MTOK = BL * MEM        # 512 memory tokens per core
VL = V // NC           # 4000 vocab per core
NEG = -1e9
EPS = 1e-5
KC = D // 128          # 4 contraction chunks of the model dim
NMB = MTOK // 128      # 4 memory-token tiles
NU = DF // 128         # 16 ffn-hidden chunks

F32 = mybir.dt.float32
AF = mybir.ActivationFunctionType
OP = mybir.AluOpType

DEFAULT_CFG = dict(
    cdt="bfloat16",     # matmul operand dtype: bfloat16 | float32r
    n_layers=L,         # layers to emit (debug)
    head=True,          # emit the vocab head (False -> output final x, debug)
    psum_dma=False,     # PSUM is not a legal DMA source on TRN2 bass
)

REGIONS = []            # (first_instr_id, label) build-time markers for analysis


def _cdt(cfg):
    return getattr(mybir.dt, cfg["cdt"])


def _np_cdt(cfg):
    return ml_dtypes.bfloat16 if cfg["cdt"] == "bfloat16" else np.float32


# ======================================================================
# device kernel builder
# ======================================================================

def build_kernel(cfg):
    CDT = _cdt(cfg)
    nc = bacc.Bacc("TRN2", target_bir_lowering=False, debug=False, num_devices=NC)
    REGIONS.clear()

    def mark(label):
        REGIONS.append((nc.next_id(), label))

    NLYR = cfg["n_layers"]

    # ---- DRAM parameters ----
    x0_d = nc.dram_tensor("x0", [TOK, D], F32, kind="ExternalInput")
    memT_d = nc.dram_tensor("memT", [128, KC * MTOK], CDT, kind="ExternalInput")
    wqk_d = nc.dram_tensor("wqk", [L, D, 2 * D], CDT, kind="ExternalInput")
    wv_d = nc.dram_tensor("wv", [L, D, D], CDT, kind="ExternalInput")
    wsao_d = nc.dram_tensor("wsao", [L, D, D], CDT, kind="ExternalInput")
    wcaq_d = nc.dram_tensor("wcaq", [L, D, D], CDT, kind="ExternalInput")
    wcak_d = nc.dram_tensor("wcak", [L, D, D], CDT, kind="ExternalInput")
    wcav_d = nc.dram_tensor("wcav", [L, D, D], CDT, kind="ExternalInput")
    wcao_d = nc.dram_tensor("wcao", [L, D, D], CDT, kind="ExternalInput")
    wf1_d = nc.dram_tensor("wf1", [L, D, DF], CDT, kind="ExternalInput")
    wf2_d = nc.dram_tensor("wf2", [L, DF, D], CDT, kind="ExternalInput")
    whead_d = nc.dram_tensor("whead", [D, V], CDT, kind="ExternalInput")
    mask_d = nc.dram_tensor("mask01", [T, T], CDT, kind="ExternalInput")
    ident_d = nc.dram_tensor("ident", [128, 128], CDT, kind="ExternalInput")

    NVB = 8           # vocab output blocks
    VBS = V // NVB    # 4000 vocab per block
    VC = 500          # psum chunk width
    if cfg["head"]:
        out_d = nc.dram_tensor("out", [NVB, TOK, VBS], CDT, kind="ExternalOutput")
    else:
        out_d = nc.dram_tensor("out", [TOK, D], F32, kind="ExternalOutput")

    with tile.TileContext(nc) as tc:
        with (
            tc.tile_pool(name="const", bufs=1) as constp,
            tc.tile_pool(name="state", bufs=1) as statep,
            tc.tile_pool(name="work", bufs=2) as work,
            tc.tile_pool(name="soft", bufs=6) as soft,
            tc.tile_pool(name="wpre", bufs=1) as wpre,
            tc.tile_pool(name="ps_a", bufs=4, space="PSUM") as ps_a,
            tc.tile_pool(name="ps_tp", bufs=2, space="PSUM") as ps_tp,
            tc.tile_pool(name="ps_att", bufs=2, space="PSUM") as ps_att,
        ):
            mark('setup')
            # ---------- DMA order: memT, wcak (unblocks kcaT matmuls ~4.5us),
            # then x0 (LN1 path), then wcav ----------
            memT = statep.tile([128, KC * MTOK], CDT, name="memT", tag="memT")
            HKM = KC * MTOK // 2
            nc.sync.dma_start(memT[:, :HKM], memT_d[:, :HKM])

            wk0 = wpre.tile([128, KC * D], CDT, name="wcak_t", tag="wcak_t",
                            bufs=2)
            nc.sync.dma_start(
                wk0[:, :KC * D // 2].rearrange("p (k n) -> p k n", k=KC // 2),
                wcak_d[0, :D // 2].rearrange("(k p) n -> p k n", p=128))
            nc.sync.dma_start(memT[:, HKM:], memT_d[:, HKM:])
            nc.sync.dma_start(
                wk0[:, KC * D // 2:].rearrange("p (k n) -> p k n", k=KC // 2),
                wcak_d[0, D // 2:].rearrange("(k p) n -> p k n", p=128))

            x_all = statep.tile([128, BL * D], F32, name="x_all", tag="x_all")
            nc.sync.dma_start(
                x_all[:].rearrange("p (b d) -> p b d", b=BL),
                x0_d[:].rearrange("(b p) d -> p b d", p=T))
            x = [x_all[:, b * D:(b + 1) * D] for b in range(BL)]

            wv0 = wpre.tile([128, KC * D], CDT, name="wcav_t", tag="wcav_t",
                            bufs=2)
            nc.sync.dma_start(
                wv0[:].rearrange("p (k n) -> p k n", k=KC),
                wcav_d[0].rearrange("(k p) n -> p k n", p=128))

            # ---------- constants ----------
            ident = constp.tile([128, 128], CDT, name="ident", tag="ident")
            nc.sync.dma_start(ident[:], ident_d[:])
            mask01 = constp.tile([T, T], CDT, name="mask01", tag="mask01")
            nc.sync.dma_start(mask01[:], mask_d[:])

            sx0 = soft.tile([128, BL], F32, name="sx0", tag="sx", bufs=3)
            for b in range(BL):
                scr0 = work.tile([T, D], CDT, name="ln_scr", tag="ln_scr")
                nc.scalar.activation(
                    scr0[:], x[b], AF.Identity, accum_out=sx0[:, b:b + 1])

            def memT_k(k):
                return memT[:, k * MTOK:(k + 1) * MTOK]

            # rsqrt magic constant for Newton iterations (no ACT table needed)
            magic = constp.tile([128, 4], mybir.dt.int32, name="magic", tag="magic")
            nc.vector.memset(magic[:], 0x5f3759df)
            ones_c = constp.tile([128, 4], CDT, name="ones_c", tag="ones_c")
            nc.vector.memset(ones_c[:], 1.0)

            hh_next = None    # per-b LN output feeding the next block's matmuls

            # ---------- helpers ----------
            def layer_norm_b(xt, b, sx):
                """Per-batch LN (gain/bias folded into next matmul) -> [T,D] CDT.

                sx [128, BL]: per-token row sums of x (from the residual
                accumulate). Variance from a table-free ACT Square pass;
                rstd via inverse-sqrt bit trick + 1 Newton step on DVE
                (bf16 hh quantization dominates the rstd error).
                """
                ssq = soft.tile([128, 1], F32, name="ln_ssq", tag="ln_ssq")
                scr = work.tile([T, D], CDT, name="ln_scr", tag="ln_scr")
                nc.scalar.activation(
                    scr[:], xt, AF.Square, accum_out=ssq[:])
                sxb = sx[:, b:b + 1]
                m2 = soft.tile([128, 1], F32, name="ln_m2", tag="ln_m2")
                nc.vector.scalar_tensor_tensor(      # m2 = (sx/D)^2
                    m2[:], sxb, 1.0 / (D * D), sxb, OP.mult, OP.mult)
                w1 = soft.tile([128, 1], F32, name="ln_w1", tag="ln_w1")
                nc.vector.tensor_scalar(             # w1 = ssq/D + eps
                    w1[:], ssq[:], 1.0 / D, EPS, OP.mult, OP.add)
                w = soft.tile([128, 1], F32, name="ln_w", tag="ln_w")
                nc.vector.tensor_tensor(w[:], w1[:], m2[:], OP.subtract)
                yb = soft.tile([128, 1], mybir.dt.int32, name="ln_yb", tag="ln_yb")
                nc.vector.tensor_scalar(
                    yb[:], w[:].bitcast(mybir.dt.int32), 1, None,
                    OP.arith_shift_right)
                y = soft.tile([128, 1], F32, name="ln_y", tag="ln_y")
                nc.vector.tensor_tensor(
                    y[:].bitcast(mybir.dt.int32), magic[:, :1], yb[:], OP.subtract)
                t1 = soft.tile([128, 1], F32, name="ln_t1", tag="ln_t1")
                nc.vector.scalar_tensor_tensor(      # t1 = y^2 * w
                    t1[:], y[:], y[:], w[:], OP.mult, OP.mult)
                nc.vector.tensor_scalar(t1[:], t1[:], -0.5, 1.5, OP.mult, OP.add)
                nc.vector.tensor_tensor(y[:], y[:], t1[:], OP.mult)
                mrstd = soft.tile([128, 1], F32, name="ln_nmr", tag="ln_nmr")
                nc.vector.scalar_tensor_tensor(      # mrstd = (sx/D) * rstd
                    mrstd[:], sxb, 1.0 / D, y[:], OP.mult, OP.mult)
                hh = work.tile([T, D], CDT, name=f"hh{b}", tag=f"hh{b}")
                for kk in range(KC):
                    nc.vector.tensor_scalar(
                        hh[:, kk * 128:(kk + 1) * 128],
                        xt[:, kk * 128:(kk + 1) * 128],
                        y[:], mrstd[:], OP.mult, OP.subtract)
                return hh

            def layer_norm_to(xt_list, sx):
                return [layer_norm_b(xt_list[b], b, sx) for b in range(BL)]

            def residual(ps_list_or_ps, b, sx):
                """x[b] += psum, accumulating row sums into sx[:, b]."""
                nc.vector.scalar_tensor_tensor(
                    x[b], ps_list_or_ps, 1.0, x[b], OP.mult, OP.add,
                    accum_out=sx[:, b:b + 1])

            def transpose_to_fm(hh_tiles):
                """[T, D] token-major tiles -> one feature-major tile [128, KC*TOK]."""
                hT = work.tile([128, KC * TOK], CDT, name="hT", tag="hT")
                for b in range(BL):
                    for k in range(KC):
                        tp = ps_tp.tile([128, 128], CDT, name="tp", tag="tp")
                        nc.tensor.transpose(
                            tp[:], hh_tiles[b][:, k * 128:(k + 1) * 128], ident[:])
                        nc.vector.tensor_copy(
                            hT[:, k * TOK + b * T:k * TOK + (b + 1) * T], tp[:])
                return hT

            def hT_k(hT, k, S=TOK):
                return hT[:, k * S:(k + 1) * S]

            def load_weight(w_dram, l, n_feat, tag, bufs=1):
                """One DMA for a [D, n_feat] weight -> [128, KC*n_feat] tile."""
                wt = wpre.tile([128, KC * n_feat], CDT, name=tag, tag=tag, bufs=bufs)
                nc.sync.dma_start(
                    wt[:].rearrange("p (k n) -> p k n", k=KC),
                    w_dram[l].rearrange("(k p) n -> p k n", p=128))
                return wt

            def w_rhs(wt, k, n_feat):
                return wt[:, k * n_feat:(k + 1) * n_feat]

            def w_lhs(wt, k, m, n_feat):
                o = k * n_feat + m * 128
                return wt[:, o:o + 128]

            def proj_fm(wt, hT, n_feat, out_tag, S=TOK, bufs=2, eng=None):
                """Feature-major projection: out slices [128, S] of (h@W).T."""
                out = work.tile([128, (n_feat // 128) * S], CDT,
                                name=out_tag, tag=out_tag, bufs=bufs)
                for m in range(n_feat // 128):
                    ps = ps_a.tile([128, S], F32, name="mm", tag="mm")
                    for k in range(KC):
                        nc.tensor.matmul(
                            ps[:], w_lhs(wt, k, m, n_feat),
                            hT_k(hT, k) if S == TOK else memT_k(k),
                            start=(k == 0), stop=(k == KC - 1))
                    if eng is nc.scalar:
                        nc.scalar.copy(out[:, m * S:(m + 1) * S], ps[:])
                    else:
                        nc.vector.tensor_copy(out[:, m * S:(m + 1) * S], ps[:])
                return out

            # ================= transformer layers =================
            def ca_kv_proj(l, wcak_t=None, wcav_t=None):
                """CA K/V projections for layer l (independent of x)."""
                if wcak_t is None:
                    wcak_t = load_weight(wcak_d, l, D, "wcak_t", bufs=2)
                kcaT = proj_fm(wcak_t, None, D, "kcaT", S=MTOK, eng=nc.scalar)
                if wcav_t is None:
                    wcav_t = load_weight(wcav_d, l, D, "wcav_t", bufs=2)
                vca = work.tile([128, NMB * D], CDT, name="vca", tag="vca", bufs=2)
                for mb in range(NMB):
                    ps = ps_a.tile([128, D], F32, name="mm", tag="mm")
                    for k in range(KC):
                        nc.tensor.matmul(
                            ps[:], memT_k(k)[:, mb * 128:(mb + 1) * 128],
                            w_rhs(wcav_t, k, D),
                            start=(k == 0), stop=(k == KC - 1))
                    nc.scalar.copy(vca[:, mb * D:(mb + 1) * D], ps[:])
                return kcaT, vca

            sx_cur = sx0
            hh_next = layer_norm_to(x, sx0)
            mark('L0.ca_kv_proj')
            kv_next = ca_kv_proj(0, wcak_t=wk0, wcav_t=wv0)
            for l in range(NLYR):
                kcaT, vca = kv_next
                mark(f'L{l}.ln1+qkv')
                # ---------- self-attention (hh_next = LN1, computed in the
                # previous block's residual loop) ----------
                hT = transpose_to_fm(hh_next)

                wqk_t = load_weight(wqk_d, l, 2 * D, "wqk_t", bufs=2)
                qkT = proj_fm(wqk_t, hT, 2 * D, "qkT")

                # v token-major: v_sb[b] [T, D] slices
                wv_t = load_weight(wv_d, l, D, "wv_t")
                v_sb = work.tile([T, BL * D], CDT, name="v_sb", tag="v_sb", bufs=1)
                for b in range(BL):
                    ps = ps_a.tile([T, D], F32, name="mm", tag="mm")
                    for k in range(KC):
                        nc.tensor.matmul(
                            ps[:], hT_k(hT, k)[:, b * T:(b + 1) * T],
                            w_rhs(wv_t, k, D),
                            start=(k == 0), stop=(k == KC - 1))
                    nc.vector.tensor_copy(v_sb[:, b * D:(b + 1) * D], ps[:])

                mark(f'L{l}.sa_attn')
                # attention per (b, h)
                attnT = work.tile([128, KC * TOK], CDT, name="attnT", tag="attnT",
                                  bufs=2)
                for b in range(BL):
                    for hp in range(H // 2):      # head pairs (h=2hp, 2hp+1)
                        mq = hp
                        tp2 = ps_tp.tile([T, 2 * T], CDT, name="tp", tag="tp")
                        p2 = soft.tile([T, 2 * T], CDT, name="p_raw", tag="p_raw", bufs=4)
                        for hi in range(2):
                            po = hi * HD
                            q_sl = qkT[po:po + HD,
                                       mq * TOK + b * T:mq * TOK + (b + 1) * T]
                            k_sl = qkT[po:po + HD,
                                       (KC + mq) * TOK + b * T:(KC + mq) * TOK + (b + 1) * T]
                            s_ps = ps_a.tile([T, T], F32, name="mm", tag="mm")
                            nc.tensor.matmul(s_ps[:], q_sl, k_sl, start=True, stop=True)
                            nc.scalar.activation(p2[:, hi * T:(hi + 1) * T], s_ps[:], AF.Exp)
                        for hi in range(2):
                            pm = soft.tile([T, T], CDT, name="pm", tag="pm")
                            r = soft.tile([T, 1], F32, name="r", tag="r")
                            nc.vector.scalar_tensor_tensor(
                                pm[:], p2[:, hi * T:(hi + 1) * T], 1.0, mask01[:],
                                OP.mult, OP.mult, accum_out=r[:])
                            rinv = soft.tile([T, 1], F32, name="rinv", tag="rinv")
                            nc.vector.reciprocal(rinv[:], r[:])
                            pn = soft.tile([T, T], CDT, name="pn", tag="pn")
                            nc.vector.tensor_scalar_mul(pn[:], pm[:], rinv[:])
                            nc.tensor.transpose(
                                tp2[:, hi * T:(hi + 1) * T], pn[:], ident[:])
                        pT = soft.tile([T, 2 * T], CDT, name="pT", tag="pT")
                        nc.vector.tensor_copy(pT[:], tp2[:])
                        a_ps = ps_att.tile([128, T], F32, name="att", tag="att")
                        for hi in range(2):
                            h = 2 * hp + hi
                            nc.tensor.matmul(
                                a_ps[hi * HD:(hi + 1) * HD, :],
                                v_sb[:, b * D + h * HD:b * D + (h + 1) * HD],
                                pT[:, hi * T:(hi + 1) * T], start=True, stop=True)
                        nc.vector.tensor_copy(
                            attnT[:, mq * TOK + b * T:mq * TOK + (b + 1) * T],
                            a_ps[:])

                if l + 1 < NLYR:
                    mark(f'L{l+1}.ca_kv_proj')
                    kv_next = ca_kv_proj(l + 1)

                mark(f'L{l}.sa_out')
                # out projection + residual + fused per-b LN2
                wsao_t = load_weight(wsao_d, l, D, "wsao_t", bufs=1)
                sx_cur = soft.tile([128, BL], F32, name="sx_sa", tag="sx", bufs=3)
                hh2 = []
                for b in range(BL):
                    y_ps = ps_a.tile([T, D], F32, name="mm", tag="mm")
                    for k in range(KC):
                        nc.tensor.matmul(
                            y_ps[:],
                            hT_k(attnT, k)[:, b * T:(b + 1) * T],
                            w_rhs(wsao_t, k, D),
                            start=(k == 0), stop=(k == KC - 1))
                    residual(y_ps[:], b, sx_cur)
                    hh2.append(layer_norm_b(x[b], b, sx_cur))

                mark(f'L{l}.ln2+q')
                # ---------- cross-attention ----------
                hT2 = transpose_to_fm(hh2)

                wcaq_t = load_weight(wcaq_d, l, D, "wcaq_t")
                qcaT = proj_fm(wcaq_t, hT2, D, "qcaT")

                mark(f'L{l}.ca_attn')
                attnC = work.tile([128, KC * TOK], CDT, name="attnC", tag="attnC",
                                  bufs=1)
                # transposed scores: sT[k_mem, q] per head via swapped
                # operands; exp lands P^T in SBUF directly (no transposes),
                # row sums via accumulated ones-matmul, normalization via
                # Pool partition_broadcast folded into the attnC copy.
                NJ = MEM // 128
                for b in range(BL):
                    for hp in range(H // 2):
                        mq = hp
                        a_ps = ps_att.tile([128, T], F32, name="att", tag="att")
                        for hi in range(2):
                            h = 2 * hp + hi
                            po = hi * HD
                            q_sl = qcaT[po:po + HD,
                                        mq * TOK + b * T:mq * TOK + (b + 1) * T]
                            s2 = ps_a.tile([128, NJ * T], F32, name="mm", tag="mm")
                            for j in range(NJ):
                                k_sl = kcaT[po:po + HD,
                                            mq * MTOK + b * MEM + j * 128:
                                            mq * MTOK + b * MEM + (j + 1) * 128]
                                nc.tensor.matmul(s2[:, j * T:(j + 1) * T],
                                                 k_sl, q_sl, start=True, stop=True)
                            pT2 = soft.tile([128, NJ * T], CDT, name="pc",
                                            tag="pc", bufs=4)
                            nc.scalar.activation(pT2[:], s2[:], AF.Exp)
                            r_ps = ps_tp.tile([1, T], F32, name="rps", tag="tp")
                            for j in range(NJ):
                                nc.tensor.matmul(
                                    r_ps[:], ones_c[:, :1],
                                    pT2[:, j * T:(j + 1) * T],
                                    start=(j == 0), stop=(j == NJ - 1))
                            rinv = soft.tile([1, T], F32, name="rinvT", tag="rinvT")
                            nc.vector.reciprocal(rinv[:], r_ps[:])
                            rb = soft.tile([HD, T], F32, name="rb", tag="rb", bufs=4)
                            nc.gpsimd.partition_broadcast(rb[:], rinv[:])
                            for j in range(NJ):
                                mbi = b * NJ + j
                                nc.tensor.matmul(
                                    a_ps[po:po + HD, :],
                                    vca[:, mbi * D + h * HD:mbi * D + (h + 1) * HD],
                                    pT2[:, j * T:(j + 1) * T],
                                    start=(j == 0), stop=(j == NJ - 1))
                            nc.vector.tensor_tensor(
                                attnC[po:po + HD,
                                      mq * TOK + b * T:mq * TOK + (b + 1) * T],
                                a_ps[po:po + HD, :], rb[:], OP.mult)

                mark(f'L{l}.ca_out')
                wcao_t = load_weight(wcao_d, l, D, "wcao_t", bufs=1)
                sx_cur = soft.tile([128, BL], F32, name="sx_ca", tag="sx", bufs=3)
                hh3 = []
                for b in range(BL):
                    yc_ps = ps_a.tile([T, D], F32, name="mm", tag="mm")
                    for k in range(KC):
                        nc.tensor.matmul(
                            yc_ps[:],
                            hT_k(attnC, k)[:, b * T:(b + 1) * T],
                            w_rhs(wcao_t, k, D),
                            start=(k == 0), stop=(k == KC - 1))
                    residual(yc_ps[:], b, sx_cur)
                    hh3.append(layer_norm_b(x[b], b, sx_cur))

                mark(f'L{l}.ln3+ffn')
                # ---------- FFN ----------
                hT3 = transpose_to_fm(hh3)

                # wf1 streamed in halves (feature cols 0:1024, 1024:2048)
                DH = DF // 2
                uT = work.tile([128, NU * TOK], CDT, name="uT", tag="uT", bufs=1)
                for half in range(2):
                    wf1h = wpre.tile([128, KC * DH], CDT, name="wf1h",
                                     tag="wf1h", bufs=2)
                    nc.sync.dma_start(
                        wf1h[:].rearrange("p (k n) -> p k n", k=KC),
                        wf1_d[l, :, half * DH:(half + 1) * DH]
                        .rearrange("(k p) n -> p k n", p=128))
                    for mm_ in range(NU // 2):
                        m = half * (NU // 2) + mm_
                        u_ps = ps_a.tile([128, TOK], F32, name="mm", tag="mm")
                        for k in range(KC):
                            nc.tensor.matmul(
                                u_ps[:], w_lhs(wf1h, k, mm_, DH), hT_k(hT3, k),
                                start=(k == 0), stop=(k == KC - 1))
                        nc.scalar.activation(
                            uT[:, m * TOK:(m + 1) * TOK], u_ps[:], AF.Gelu)

                wf2h = []
                for half in range(2):
                    w2 = wpre.tile([128, (NU // 2) * D], CDT, name="wf2h",
                                   tag=f"wf2h{half}", bufs=1)
                    nc.sync.dma_start(
                        w2[:].rearrange("p (k n) -> p k n", k=NU // 2),
                        wf2_d[l, half * DH:(half + 1) * DH]
                        .rearrange("(k p) n -> p k n", p=128))
                    wf2h.append(w2)
                sx_cur = soft.tile([128, BL], F32, name="sx_f", tag="sx", bufs=3)
                hh_next = []
                for b in range(BL):
                    yf_ps = ps_a.tile([T, D], F32, name="mm", tag="mm")
                    for m in range(NU):
                        nc.tensor.matmul(
                            yf_ps[:],
                            uT[:, m * TOK + b * T:m * TOK + (b + 1) * T],
                            wf2h[m // (NU // 2)][:, (m % (NU // 2)) * D:
                                                 (m % (NU // 2) + 1) * D],
                            start=(m == 0), stop=(m == NU - 1))
                    residual(yf_ps[:], b, sx_cur)
                    hh_next.append(layer_norm_b(x[b], b, sx_cur))

            mark('head')
            # ================= final LN + head =================
            if not cfg["head"]:
                for b in range(BL):
                    nc.sync.dma_start(out_d[b * T:(b + 1) * T, :], x[b])
            else:
                xfT = transpose_to_fm(hh_next)
                # 2MB weight chunks; final chunk split finer so the last
                # output DMAs drain while earlier compute still runs
                widths = [2000] * 15 + [1000, 500, 500]
                vo = 0
                for gwc, WCW in enumerate(widths):
                    wh = wpre.tile([128, KC * WCW], CDT, name="wh_t",
                                   tag="wh_t", bufs=2)
                    nc.sync.dma_start(
                        wh[:].rearrange("p (k v) -> p k v", k=KC),
                        whead_d[:, vo:vo + WCW]
                        .rearrange("(k p) v -> p k v", p=128))
                    vb, wcc = vo // VBS, (vo % VBS)
                    nsub = WCW // VC
                    for tt in range(TOK // T):
                        stg = work.tile([T, WCW], CDT, name=f"hstg{tt}",
                                        tag=f"hstg{tt}", bufs=2)
                        for sub in range(nsub):
                            o_ps = ps_a.tile([T, VC], F32, name="mm", tag="mm")
                            for k in range(KC):
                                nc.tensor.matmul(
                                    o_ps[:],
                                    hT_k(xfT, k)[:, tt * T:(tt + 1) * T],
                                    wh[:, k * WCW + sub * VC:
                                       k * WCW + (sub + 1) * VC],
                                    start=(k == 0), stop=(k == KC - 1))
                            dst = stg[:, sub * VC:(sub + 1) * VC]
                            if (gwc + tt + sub) % 2 == 0:
                                nc.scalar.copy(dst, o_ps[:])
                            else:
                                nc.vector.tensor_copy(dst, o_ps[:])
                        nc.sync.dma_start(
                            out_d[vb, tt * T:(tt + 1) * T, wcc:wcc + WCW],
                            stg[:])
                    vo += WCW

    nc.compile()
    return nc


# ======================================================================
# host side
# ======================================================================

def _prep_inputs(cfg, inputs):
    """Fold params, embed tokens, build the 8 per-core input maps."""
    npdt = _np_cdt(cfg)
    f32 = np.float32
    tok_emb = np.asarray(inputs["tok_emb"], f32)
    pos_emb = np.asarray(inputs["pos_emb"], f32)
    targets = np.asarray(inputs["targets"])
    memory = np.asarray(inputs["memory"], f32)

    inp = np.concatenate(
        [np.full((B, 1), BOS, dtype=targets.dtype), targets[:, :-1]], axis=1)
    x0 = tok_emb[inp] + pos_emb[:T][None]          # [B, T, D] f32
    x0 = np.ascontiguousarray(x0, f32)

    scale = 1.0 / np.sqrt(HD)

    def fold(w, g):
        return np.asarray(g, f32)[:, None] * np.asarray(w, f32)

    wqk = np.empty((L, D, 2 * D), f32)
    wv = np.empty((L, D, D), f32)
    wsao = np.empty((L, D, D), f32)
    wcaq = np.empty((L, D, D), f32)
    wcak = np.empty((L, D, D), f32)
    wcav = np.empty((L, D, D), f32)
    wcao = np.empty((L, D, D), f32)
    wf1 = np.empty((L, D, DF), f32)
    wf2 = np.empty((L, DF, D), f32)
    for l in range(L):
        wqkv = fold(inputs["sa_qkv_w"][l], inputs["ln1_g"][l])
        wqk[l, :, :D] = wqkv[:, :D] * scale           # fold softmax scale into Q
        wqk[l, :, D:] = wqkv[:, D:2 * D]
        wv[l] = wqkv[:, 2 * D:]
        wsao[l] = np.asarray(inputs["sa_out_w"][l], f32)
        wcaq[l] = fold(inputs["ca_q_w"][l], inputs["ln2_g"][l]) * scale
        ckv = np.asarray(inputs["ca_kv_w"][l], f32)
        wcak[l] = ckv[:, :D]
        wcav[l] = ckv[:, D:]
        wcao[l] = np.asarray(inputs["ca_out_w"][l], f32)
        wf1[l] = fold(inputs["ffn1_w"][l], inputs["ln3_g"][l])
        wf2[l] = np.asarray(inputs["ffn2_w"][l], f32)

    whead = np.asarray(inputs["normf_g"], f32)[:, None] * np.asarray(inputs["out_w"], f32)

    whead_b = whead.astype(npdt)
    mask01 = np.tril(np.ones((T, T), f32))
    ident = np.eye(128, dtype=f32)

    shared = {
        "wqk": wqk.astype(npdt), "wv": wv.astype(npdt),
        "wsao": wsao.astype(npdt), "wcaq": wcaq.astype(npdt),
        "wcak": wcak.astype(npdt), "wcav": wcav.astype(npdt),
        "wcao": wcao.astype(npdt),
        "wf1": wf1.astype(npdt), "wf2": wf2.astype(npdt),
        "mask01": mask01.astype(npdt), "ident": ident.astype(npdt),
    }
    in_maps = []
    for c in range(NC):
        m = dict(shared)
        m["x0"] = np.ascontiguousarray(
            x0[c * BL:(c + 1) * BL].reshape(TOK, D))
        # memT[p, k*MTOK + m] = mem[m, k*128 + p]  (feature-major, CDT)
        mc = memory[c * BL:(c + 1) * BL].reshape(MTOK, D)
        m["memT"] = np.ascontiguousarray(
            mc.reshape(MTOK, KC, 128).transpose(2, 1, 0)
            .reshape(128, KC * MTOK).astype(npdt))
        m["whead"] = whead_b
        in_maps.append(m)
    return in_maps


def _biases_trivial(inputs):
    for k in ("sa_qkv_b", "sa_out_b", "ca_q_b", "ca_kv_b", "ca_out_b",
              "ffn1_b", "ffn2_b", "ln1_b", "ln2_b", "ln3_b", "normf_b"):
        if np.any(np.asarray(inputs[k])):
            return False
    return True


def _numpy_fallback(inputs):
    """Exact (slow) host fallback, used only if bias inputs are nonzero."""
    try:
        from scipy.special import erf
    except ImportError:
        import math
        erf = np.vectorize(math.erf)

    f = {k: (np.asarray(v) if np.asarray(v).dtype == np.int64
             else np.asarray(v, np.float32)) for k, v in inputs.items()}

    def ln(x, g, b):
        m = x.mean(-1, keepdims=True)
        v = ((x - m) ** 2).mean(-1, keepdims=True)
        return (x - m) / np.sqrt(v + EPS) * g + b

    def split(t):
        return t.reshape(t.shape[0], t.shape[1], H, HD).transpose(0, 2, 1, 3)

    def merge(t):
        return t.transpose(0, 2, 1, 3).reshape(t.shape[0], t.shape[2], D)

    def softmax(s):
        s = s - s.max(-1, keepdims=True)
        e = np.exp(s)
        return e / e.sum(-1, keepdims=True)

    targets = f["targets"]
    inp = np.concatenate(
        [np.full((B, 1), BOS, dtype=targets.dtype), targets[:, :-1]], axis=1)
    x = f["tok_emb"][inp] + f["pos_emb"][:T][None]
    causal = np.tril(np.ones((T, T), bool))
    scale = 1.0 / np.sqrt(HD)
    for l in range(L):
        h = ln(x, f["ln1_g"][l], f["ln1_b"][l])
        qkv = h @ f["sa_qkv_w"][l] + f["sa_qkv_b"][l]
        q, k, v = np.split(qkv, 3, axis=-1)
        q, k, v = split(q), split(k), split(v)
        s = np.einsum('bhqd,bhkd->bhqk', q, k) * scale
        a = softmax(np.where(causal, s, NEG))
        x = x + merge(np.einsum('bhqk,bhkd->bhqd', a, v)) @ f["sa_out_w"][l] + f["sa_out_b"][l]
        h = ln(x, f["ln2_g"][l], f["ln2_b"][l])
        q = split(h @ f["ca_q_w"][l] + f["ca_q_b"][l])
        kv = f["memory"] @ f["ca_kv_w"][l] + f["ca_kv_b"][l]
        k, v = np.split(kv, 2, axis=-1)
        k, v = split(k), split(v)
        s = np.einsum('bhqd,bhkd->bhqk', q, k) * scale
        a = softmax(s)
        x = x + merge(np.einsum('bhqk,bhkd->bhqd', a, v)) @ f["ca_out_w"][l] + f["ca_out_b"][l]
        h = ln(x, f["ln3_g"][l], f["ln3_b"][l])
        g = h @ f["ffn1_w"][l] + f["ffn1_b"][l]
        g = 0.5 * g * (1 + erf(g / np.sqrt(2.0)))
        x = x + g @ f["ffn2_w"][l] + f["ffn2_b"][l]
    x = ln(x, f["normf_g"], f["normf_b"])
    return (x @ f["out_w"] + f["out_b"]).astype(np.float32)


_BUILT = {}


def get_built(cfg=None):
    cfg = dict(DEFAULT_CFG, **(cfg or {}))
    cfg_key = tuple(sorted(cfg.items()))
    if cfg_key not in _BUILT:
        _BUILT[cfg_key] = build_kernel(cfg)
    return _BUILT[cfg_key], cfg


def run_device(inputs, cfg=None):
    nc, cfg = get_built(cfg)
    in_maps = _prep_inputs(cfg, inputs)
    res = run_bass_kernel_spmd(nc, in_maps, core_ids=list(range(NC)))
    outs = [res.results[c]["out"] for c in range(NC)]
    if not cfg["head"]:
        # debug: final residual stream per core -> [B, T, D]
        return np.concatenate([o.reshape(BL, T, D) for o in outs], axis=0)
    # per-core out: [8 vocab-blocks, 256 tok, 4000] for that core's 2 batches
    logits = np.empty((B, T, V), np.float32)
    for c in range(NC):
        oc = outs[c].transpose(1, 0, 2).reshape(BL, T, V).astype(np.float32)
        logits[c * BL:(c + 1) * BL] = oc
    out_b = np.asarray(inputs["out_b"], np.float32)
    normf_b = np.asarray(inputs["normf_b"], np.float32)
    bias = normf_b @ np.asarray(inputs["out_w"], np.float32) + out_b
    if np.any(bias):
        logits = logits + bias
    return logits


def kernel(**inputs) -> np.ndarray:
    if not _biases_trivial(inputs):
        return _numpy_fallback(inputs)
    return run_device(inputs)

